# revision 1
# baseline (speedup 1.0000x reference)
"""BiQRNN (fo-pooling) Trainium2 kernel.

Data-parallel over batch across 8 NeuronCores (2 batch rows per core).
Per direction: g = W @ x (fp32r matmuls, weights stationary, gates on
partitions / time on free), ACT tanh/sigmoid out of PSUM, then the
hardware tensor_tensor_scan runs h_t = a_t*h_{t-1} + c_t along the free
(time) axis, chunk-chained via the `initial` operand. The backward
direction is the same forward routine run on a host-reversed copy of X.
"""

import numpy as np

import concourse.bacc as bacc
import concourse.mybir as mybir
import concourse.tile as tile
from concourse import bass_utils

SEQ, BATCH, D_IN, HID = 2048, 16, 512, 512
NCORES = 8
BPC = BATCH // NCORES  # batch rows per core

f32 = mybir.dt.float32
f32r = mybir.dt.float32r
Alu = mybir.AluOpType
Act = mybir.ActivationFunctionType


def build_nc(seq=SEQ, bpc=BPC, t_chunk=512):
    kt = D_IN // 128   # contraction tiles
    ht_n = HID // 128  # output tiles per gate
    mt = 3 * ht_n      # m tiles (z, f, o)
    tok = bpc * seq
    nch = seq // t_chunk
    T = t_chunk

    nc = bacc.Bacc("TRN2", target_bir_lowering=False, debug=False)
    XT = nc.dram_tensor("xt", [2, kt, 128, tok], f32r, kind="ExternalInput")
    WT = nc.dram_tensor("wt", [2, kt, 128, 3 * HID], f32r, kind="ExternalInput")
    BIAS = nc.dram_tensor("bias", [2, 128, mt], f32, kind="ExternalInput")
    Y = nc.dram_tensor("y", [2, ht_n, 128, tok], f32, kind="ExternalOutput")

    with tile.TileContext(nc) as tc:
        with (
            tc.tile_pool(name="wpool", bufs=1) as wpool,
            tc.tile_pool(name="bpool", bufs=1) as bpool,
            tc.tile_pool(name="rhs_pool", bufs=4) as rhs_pool,
            tc.tile_pool(name="ps_pool", bufs=8, space="PSUM") as ps_pool,
            tc.tile_pool(name="gate_pool", bufs=6) as gate_pool,
            tc.tile_pool(name="h_pool", bufs=10) as h_pool,
            tc.tile_pool(name="y_pool", bufs=6) as y_pool,
        ):
            w_sb = [[None] * kt for _ in range(2)]
            b_sb = [None, None]

            def load_w_tile(d, k, split=False):
                w = wpool.tile([128, 3 * HID], f32r, name=f"w_{d}_{k}")
                if split:
                    # piece-DMAs land on distinct queues and move in
                    # parallel, so the first matmul's weights arrive sooner
                    q = 3 * HID // 2
                    for p in range(2):
                        nc.sync.dma_start(
                            w[:, p * q : (p + 1) * q], WT.ap()[d, k, :, p * q : (p + 1) * q]
                        )
                else:
                    nc.sync.dma_start(w[:], WT.ap()[d, k])
                w_sb[d][k] = w

            def load_bias(d, eng=None):
                bt = bpool.tile([128, mt], f32, name=f"b_{d}")
                (eng or nc.sync).dma_start(bt[:], BIAS.ap()[d])
                b_sb[d] = bt

            # Startup: Sync's serial DMA-issue cost (~0.6us each) dominates
            # first-matmul latency, so split the issue stream across two
            # engines — Sync issues weight pieces while GpSimd issues the
            # first chunk's rhs and the bias in parallel. The first weight
            # tile is 3-way split (2 pieces on Sync + 1 on GpSimd) so its
            # transfer parallelizes without serializing more issues.
            T_FIRST = T // 2
            first_rhs = rhs_pool.tile([128, kt, T], f32r, name="rhs")
            third = 3 * HID // 3
            w0 = wpool.tile([128, 3 * HID], f32r, name="w_0_0")
            nc.sync.dma_start(w0[:, :third], WT.ap()[0, 0, :, :third])
            nc.sync.dma_start(w0[:, third : 2 * third], WT.ap()[0, 0, :, third : 2 * third])
            nc.gpsimd.dma_start(w0[:, 2 * third :], WT.ap()[0, 0, :, 2 * third :])
            w_sb[0][0] = w0
            nc.gpsimd.dma_start(first_rhs[:, 0, :T_FIRST], XT.ap()[0, 0, :, 0:T_FIRST])
            load_w_tile(0, 1, split=True)
            nc.gpsimd.dma_start(first_rhs[:, 1, :T_FIRST], XT.ap()[0, 1, :, 0:T_FIRST])
            load_w_tile(0, 2, split=True)
            nc.gpsimd.dma_start(first_rhs[:, 2, :T_FIRST], XT.ap()[0, 2, :, 0:T_FIRST])
            load_w_tile(0, 3, split=True)
            nc.gpsimd.dma_start(first_rhs[:, 3, :T_FIRST], XT.ap()[0, 3, :, 0:T_FIRST])
            load_bias(0, eng=nc.gpsimd)
            for d in range(2):
                for b in range(bpc):
                    hprev = None
                    # grow-in at the start (first matmul starts on ~700KB of
                    # DMA instead of 4MB); taper at the end so the post-matmul
                    # serial chain (stt -> scan -> y -> dma) stays short
                    if d == 0 and b == 0:
                        chunks = [T // 2, T // 2] + [T] * (nch - 1)
                    elif d == 1 and b == bpc - 1:
                        chunks = [T] * (nch - 1) + [T // 2, T // 2]
                    else:
                        chunks = [T] * nch
                    t0 = b * seq
                    for ci, tc_len in enumerate(chunks):
                        first_chunk = d == 0 and b == 0 and ci == 0
                        if first_chunk:
                            rhs = first_rhs
                        else:
                            rhs = rhs_pool.tile([128, kt, T], f32r, name="rhs")
                            for k in range(kt):
                                nc.sync.dma_start(
                                    rhs[:, k, :tc_len], XT.ap()[d, k, :, t0 : t0 + tc_len]
                                )
                        if d == 0 and b == 1:
                            # bw-direction constants: one tile per chunk,
                            # spread out so no single rhs prefetch queues
                            # behind a 3MB weight burst.
                            for k in range(ci * kt // nch, min((ci + 1) * kt // nch, kt)):
                                load_w_tile(1, k)
                            if ci == nch - 1:
                                load_bias(1)
                        hcur = [None] * ht_n
                        if first_chunk:
                            # k-outer over 4 PSUM tiles: matmuls start as soon
                            # as the first k-slice of weights+X arrives instead
                            # of waiting for all 4MB.
                            ps_first = [ps_pool.tile([128, T], f32, name="ps") for _ in range(4)]
                            for k in range(kt):
                                for m in range(4):
                                    nc.tensor.matmul(
                                        ps_first[m][:, :tc_len],
                                        w_sb[d][k][:, m * 128 : (m + 1) * 128],
                                        rhs[:, k, :tc_len],
                                        start=(k == 0),
                                        stop=(k == kt - 1),
                                    )
                        for hti in range(ht_n):
                            acts = []
                            for g in range(3):
                                m = g * ht_n + hti
                                if first_chunk and m < 4:
                                    ps = ps_first[m]
                                else:
                                    ps = ps_pool.tile([128, T], f32, name="ps")
                                    for k in range(kt):
                                        nc.tensor.matmul(
                                            ps[:, :tc_len],
                                            w_sb[d][k][:, m * 128 : (m + 1) * 128],
                                            rhs[:, k, :tc_len],
                                            start=(k == 0),
                                            stop=(k == kt - 1),
                                        )
                                gt = gate_pool.tile([128, T], f32, name=("zt", "at", "ot")[g])
                                nc.scalar.activation(
                                    gt[:, :tc_len],
                                    ps[:, :tc_len],
                                    Act.Tanh if g == 0 else Act.Sigmoid,
                                    bias=b_sb[d][:, m : m + 1],
                                    scale=-1.0 if g == 1 else 1.0,
                                )
                                acts.append(gt)
                            zt, at, ot = acts
                            cp = gate_pool.tile([128, T], f32, name="cp")
                            # cp = (a - 1) * z = -c
                            nc.vector.scalar_tensor_tensor(
                                cp[:, :tc_len], at[:, :tc_len], 1.0, zt[:, :tc_len],
                                op0=Alu.subtract, op1=Alu.mult,
                            )
                            h = h_pool.tile([128, T], f32, name="h")
                            init = 0.0 if ci == 0 else hprev[hti]
                            # h_t = a_t * h_{t-1} - cp_t = a_t*h_{t-1} + (1-a_t)*z_t
                            nc.vector.tensor_tensor_scan(
                                h[:, :tc_len], at[:, :tc_len], cp[:, :tc_len], init,
                                op0=Alu.mult, op1=Alu.subtract,
                            )
                            hcur[hti] = h[:, tc_len - 1 : tc_len]
                            yt = y_pool.tile([128, T], f32, name="yt")
                            nc.gpsimd.tensor_tensor(
                                yt[:, :tc_len], ot[:, :tc_len], h[:, :tc_len], op=Alu.mult
                            )
                            nc.sync.dma_start(Y.ap()[d, hti, :, t0 : t0 + tc_len], yt[:, :tc_len])
                        hprev = hcur
                        t0 += tc_len
    nc.compile()
    return nc


def prep_inputs(X, W_fw, b_fw, W_bw, b_bw):
    """Host-side shard/transpose. Returns per-core in_maps."""
    kt = D_IN // 128
    ht_n = HID // 128
    mt = 3 * ht_n

    WT = np.empty((2, kt, 128, 3 * HID), np.float32)
    BIAS = np.empty((2, 128, mt), np.float32)
    for d, (W, bvec) in enumerate(((W_fw, b_fw), (W_bw, b_bw))):
        WT[d] = np.ascontiguousarray(W.T).reshape(kt, 128, 3 * HID)
        bm = bvec.reshape(mt, 128).T.copy()  # [128, mt]
        bm[:, ht_n : 2 * ht_n] *= -1.0  # f-gate bias negated (a = sigmoid(-g - b))
        BIAS[d] = bm

    # one big [S,B,D] -> [D,B,S] transpose, then per-core block copies
    XTa = np.ascontiguousarray(np.transpose(X, (2, 1, 0))).reshape(kt, 128, BATCH, SEQ)
    in_maps = []
    for c in range(NCORES):
        xt = np.empty((2, kt, 128, BPC, SEQ), np.float32)
        blk = XTa[:, :, c * BPC : (c + 1) * BPC, :]
        xt[0] = blk
        xt[1] = blk[..., ::-1]
        in_maps.append({"xt": xt.reshape(2, kt, 128, BPC * SEQ), "wt": WT, "bias": BIAS})
    return in_maps


def assemble_output(results):
    """results: list of per-core {'y': [2, ht, 128, tok]} -> [SEQ, BATCH, 2*HID]."""
    out = np.empty((SEQ, BATCH, 2 * HID), np.float32)
    for c in range(NCORES):
        Yc = results[c]["y"]
        for b in range(BPC):
            gb = c * BPC + b
            yf = Yc[0, :, :, b * SEQ : (b + 1) * SEQ].reshape(HID, SEQ)
            yb = Yc[1, :, :, b * SEQ : (b + 1) * SEQ].reshape(HID, SEQ)
            out[:, gb, :HID] = yf.T
            out[:, gb, HID:] = yb.T[::-1]
    return out


_NC_CACHE = {}


def _get_nc():
    if "nc" not in _NC_CACHE:
        _NC_CACHE["nc"] = build_nc()
    return _NC_CACHE["nc"]


def kernel(X, W_fw, b_fw, W_bw, b_bw, trace=False):
    X = np.asarray(X, np.float32)
    nc = _get_nc()
    in_maps = prep_inputs(
        X,
        np.asarray(W_fw, np.float32),
        np.asarray(b_fw, np.float32),
        np.asarray(W_bw, np.float32),
        np.asarray(b_bw, np.float32),
    )
    res = bass_utils.run_bass_kernel_spmd(
        nc, in_maps, core_ids=list(range(NCORES)), trace=trace
    )
    out = assemble_output(res.results)
    if trace:
        kernel.last_results = res
    return out



# revision 4
# speedup vs baseline: 1.0299x; 1.0299x over previous
"""BiQRNN (fo-pooling) Trainium2 kernel, v2 — all-bf16 dataflow.

Data-parallel over batch across 8 NeuronCores (2 batch rows per core).
Per direction: g = W @ x with bf16 weights/activations (fp32 PSUM accum,
T=1024 moving columns per matmul -> half the instruction count of fp32r
at the same 1 col/cycle PE rate), ACT tanh/sigmoid out of PSUM into bf16
gates, DVE tensor_tensor_scan (fp32 internal state) for
h_t = a_t*h_{t-1} + (1-a_t)*z_t chained across chunks, y = o*h on GpSimd,
Y stored bf16 and upcast on host. The backward direction runs the same
forward routine on a host-reversed copy of X.

Startup: the first chunk is 256 columns and its weight/X dependencies are
issued first, spread across five engine DMA queues, so the PE stream
starts ~6us in instead of waiting behind the bulk prefetch. The last
block tapers (1024/768/256) to shorten the post-matmul drain chain.
"""

import numpy as np
from ml_dtypes import bfloat16

import concourse.bacc as bacc
import concourse.mybir as mybir
import concourse.tile as tile
from concourse import bass_utils

SEQ, BATCH, D_IN, HID = 2048, 16, 512, 512
NCORES = 8
BPC = BATCH // NCORES  # batch rows per core

f32 = mybir.dt.float32
bf16 = mybir.dt.bfloat16
Alu = mybir.AluOpType
Act = mybir.ActivationFunctionType

KT = D_IN // 128   # contraction tiles
HT = HID // 128    # h tiles per gate
MT = 3 * HT        # m tiles (z, f, o)
T = 1024           # steady-state chunk (max bf16 moving operand)
T0 = 256           # taper chunk at stream head/tail


def build_nc():
    nc = bacc.Bacc("TRN2", target_bir_lowering=False, debug=False)
    XT = nc.dram_tensor("xt", [2, KT, 128, BPC * SEQ], bf16, kind="ExternalInput")
    WT = nc.dram_tensor("wt", [2, KT, 128, 3 * HID], bf16, kind="ExternalInput")
    BIAS = nc.dram_tensor("bias", [2, 128, MT], f32, kind="ExternalInput")
    Y = nc.dram_tensor("y", [2, HT, 128, BPC * SEQ], bf16, kind="ExternalOutput")

    with tile.TileContext(nc) as tc:
        with (
            tc.tile_pool(name="wpool", bufs=1) as wpool,
            tc.tile_pool(name="bpool", bufs=1) as bpool,
            tc.tile_pool(name="rhs_pool", bufs=2) as rhs_pool,
            tc.tile_pool(name="ps_pool", bufs=4, space="PSUM") as ps_pool,
            tc.tile_pool(name="gate_pool", bufs=9) as gate_pool,
            tc.tile_pool(name="h_pool", bufs=6) as h_pool,
            tc.tile_pool(name="y_pool", bufs=4) as y_pool,
        ):
            w_sb = [[None] * KT for _ in range(2)]
            b_sb = [None, None]

            def load_w(d, k, eng):
                w = wpool.tile([128, 3 * HID], bf16, name=f"w_{d}_{k}")
                eng.dma_start(w[:], WT.ap()[d, k])
                w_sb[d][k] = w

            def load_bias(d, eng):
                bt = bpool.tile([128, MT], f32, name=f"b_{d}")
                eng.dma_start(bt[:], BIAS.ap()[d])
                b_sb[d] = bt

            def new_rhs():
                return rhs_pool.tile([128, KT, SEQ], bf16, name="rhs")

            def load_rhs(t, d, b, eng, k_lo=0, k_hi=KT, c0=0, c1=SEQ):
                for k in range(k_lo, k_hi):
                    eng.dma_start(
                        t[:, k, c0:c1], XT.ap()[d, k, :, b * SEQ + c0 : b * SEQ + c1]
                    )

            # --- startup: first-chunk deps first, spread across queues ---
            # first chunk needs W[d0, all k] + rhs(d0,b0)[:, :, :T0]; the
            # rhs tails and everything else follow behind.
            rhs0 = new_rhs()
            load_rhs(rhs0, 0, 0, nc.sync, k_lo=0, k_hi=1, c1=T0)
            load_rhs(rhs0, 0, 0, nc.gpsimd, k_lo=1, k_hi=2, c1=T0)
            load_rhs(rhs0, 0, 0, nc.scalar, k_lo=2, k_hi=3, c1=T0)
            load_rhs(rhs0, 0, 0, nc.gpsimd, k_lo=3, k_hi=4, c1=T0)
            load_w(0, 0, nc.sync)
            load_w(0, 1, nc.gpsimd)
            load_w(0, 2, nc.scalar)
            load_w(0, 3, nc.sync)
            load_bias(0, nc.scalar)
            load_bias(1, nc.scalar)
            load_rhs(rhs0, 0, 0, nc.sync, k_lo=0, k_hi=1, c0=T0)
            load_rhs(rhs0, 0, 0, nc.gpsimd, k_lo=1, k_hi=2, c0=T0)
            load_rhs(rhs0, 0, 0, nc.scalar, k_lo=2, k_hi=3, c0=T0)
            load_rhs(rhs0, 0, 0, nc.gpsimd, k_lo=3, k_hi=4, c0=T0)

            rhs_next = [None]
            blocks = [(0, 0), (0, 1), (1, 0), (1, 1)]
            for bi, (d, b) in enumerate(blocks):
                if bi == 0:
                    rhs = rhs0
                    chunks = [T0, SEQ - T - T0, T]
                else:
                    rhs = rhs_next[0]
                if bi + 1 < len(blocks):
                    dn, bn = blocks[bi + 1]
                    rhs_next[0] = new_rhs()
                    load_rhs(rhs_next[0], dn, bn, nc.sync)
                if bi == len(blocks) - 1:
                    chunks = [T, SEQ - T - T0, T0]
                elif bi > 0:
                    chunks = [T, SEQ - T]

                hprev = None
                t0 = 0
                for ci, tl in enumerate(chunks):
                    if bi == 0:
                        # trickle the bw-direction constants in while the
                        # fw stream runs; gpsimd has slack
                        for k in range(ci * 2, min(ci * 2 + 2, KT)):
                            load_w(1, k, nc.gpsimd)
                    hcur = [None] * HT
                    for hti in range(HT):
                        acts = []
                        for g in range(3):
                            m = g * HT + hti
                            ps = ps_pool.tile([128, T], f32, name="ps")
                            # ISA caps one matmul at 512 psum columns (one
                            # bank), so fill the 1024-wide tile in halves
                            for s0 in range(0, tl, 512):
                                sl = min(512, tl - s0)
                                for k in range(KT):
                                    nc.tensor.matmul(
                                        ps[:, s0 : s0 + sl],
                                        w_sb[d][k][:, m * 128 : (m + 1) * 128],
                                        rhs[:, k, t0 + s0 : t0 + s0 + sl],
                                        start=(k == 0),
                                        stop=(k == KT - 1),
                                    )
                            gt = gate_pool.tile(
                                [128, T], bf16, name=("zt", "at", "ot")[g]
                            )
                            nc.scalar.activation(
                                gt[:, :tl],
                                ps[:, :tl],
                                Act.Tanh if g == 0 else Act.Sigmoid,
                                bias=b_sb[d][:, m : m + 1],
                                scale=-1.0 if g == 1 else 1.0,
                            )
                            acts.append(gt)
                        zt, at, ot = acts
                        cp = gate_pool.tile([128, T], bf16, name="cp")
                        # cp = (a - 1) * z = -c
                        nc.vector.scalar_tensor_tensor(
                            cp[:, :tl], at[:, :tl], 1.0, zt[:, :tl],
                            op0=Alu.subtract, op1=Alu.mult,
                        )
                        h = h_pool.tile([128, T], bf16, name="h")
                        init = 0.0 if ci == 0 else hprev[hti]
                        # h_t = a_t * h_{t-1} - cp_t
                        nc.vector.tensor_tensor_scan(
                            h[:, :tl], at[:, :tl], cp[:, :tl], init,
                            op0=Alu.mult, op1=Alu.subtract,
                        )
                        hcur[hti] = h[:, tl - 1 : tl]
                        yt = y_pool.tile([128, T], bf16, name="yt")
                        nc.gpsimd.tensor_tensor(
                            yt[:, :tl], ot[:, :tl], h[:, :tl], op=Alu.mult
                        )
                        nc.sync.dma_start(
                            Y.ap()[d, hti, :, b * SEQ + t0 : b * SEQ + t0 + tl],
                            yt[:, :tl],
                        )
                    hprev = hcur
                    t0 += tl
    nc.compile()
    return nc


def prep_inputs(X, W_fw, b_fw, W_bw, b_bw):
    """Host-side shard/transpose/bf16-cast. Returns per-core in_maps."""
    WTa = np.empty((2, KT, 128, 3 * HID), bfloat16)
    BIAS = np.empty((2, 128, MT), np.float32)
    for d, (W, bvec) in enumerate(((W_fw, b_fw), (W_bw, b_bw))):
        WTa[d] = np.ascontiguousarray(W.T).reshape(KT, 128, 3 * HID).astype(bfloat16)
        bm = bvec.reshape(MT, 128).T.copy()  # [128, MT]
        bm[:, HT : 2 * HT] *= -1.0  # f-gate bias negated (a = sigmoid(-g - b))
        BIAS[d] = bm

    # one big [S,B,D] -> [D,B,S] transpose + bf16 cast, then per-core blocks
    XTa = (
        np.ascontiguousarray(np.transpose(X, (2, 1, 0)))
        .astype(bfloat16)
        .reshape(KT, 128, BATCH, SEQ)
    )
    in_maps = []
    for c in range(NCORES):
        xt = np.empty((2, KT, 128, BPC, SEQ), bfloat16)
        blk = XTa[:, :, c * BPC : (c + 1) * BPC, :]
        xt[0] = blk
        xt[1] = blk[..., ::-1]
        in_maps.append(
            {"xt": xt.reshape(2, KT, 128, BPC * SEQ), "wt": WTa, "bias": BIAS}
        )
    return in_maps


def assemble_output(results):
    """results: list of per-core {'y': [2, HT, 128, tok]} -> [SEQ, BATCH, 2*HID]."""
    out = np.empty((SEQ, BATCH, 2 * HID), np.float32)
    for c in range(NCORES):
        Yc = np.asarray(results[c]["y"]).astype(np.float32)
        for b in range(BPC):
            gb = c * BPC + b
            yf = Yc[0, :, :, b * SEQ : (b + 1) * SEQ].reshape(HID, SEQ)
            yb = Yc[1, :, :, b * SEQ : (b + 1) * SEQ].reshape(HID, SEQ)
            out[:, gb, :HID] = yf.T
            out[:, gb, HID:] = yb.T[::-1]
    return out


_NC_CACHE = {}


def _get_nc():
    if "nc" not in _NC_CACHE:
        _NC_CACHE["nc"] = build_nc()
    return _NC_CACHE["nc"]


def kernel(X, W_fw, b_fw, W_bw, b_bw, trace=False):
    X = np.asarray(X, np.float32)
    nc = _get_nc()
    in_maps = prep_inputs(
        X,
        np.asarray(W_fw, np.float32),
        np.asarray(b_fw, np.float32),
        np.asarray(W_bw, np.float32),
        np.asarray(b_bw, np.float32),
    )
    res = bass_utils.run_bass_kernel_spmd(
        nc, in_maps, core_ids=list(range(NCORES)), trace=trace
    )
    out = assemble_output(res.results)
    if trace:
        kernel.last_results = res
    return out
